# revision 1
# baseline (speedup 1.0000x reference)
"""GQA kernel for 8x TRN2 NeuronCores (Bass/Tile), DP2 x TP4 sharding.

Layout strategy (per core; batch b = core//4, shard t = core%4):
  - x fed transposed (feature-major) xT [D, S]; projections emit token-major
    q/k/v and feature-major gate^T via PE matmuls.
  - rmsnorm+rope token-major (free-dim reductions), then PE-transpose q,k to
    feature-major for attention.
  - scores^T [k,128 x q,512] blocks = kT.T @ qT (K=64); exp on ScalarE; causal
    handled by block skip + 0/1 mask multiplies on mixed blocks only.
  - ctx^T accumulated feature-major with v_ext=[v|ones] so softmax sums come
    free as psum row 64; normalize via reciprocal + DMA partition-broadcast.
  - out projection token-major with ctxg as stationary operand; partial
    [S, D] outputs summed across the 4 TP shards on host.
Local head order is interleaved (0,4,1,5,2,6,3,7) so transposed q tiles put a
g0 head on partitions 0-63 and a g1 head on 64-127, matching kT/gate/Wo
layouts without any cross-partition moves.
"""
import sys

if "/opt/trn_rl_repo" not in sys.path:
    sys.path.insert(0, "/opt/trn_rl_repo")

import numpy as np

import concourse.bass as bass
import concourse.mybir as mybir
import concourse.tile as tile
from concourse import bacc
from concourse.bass_utils import run_bass_kernel_spmd

B, S, D = 2, 2048, 2048
H, G, HD = 32, 8, 64
EPS = 1e-6
NCORES = 8
NT = S // 128          # 16 s-tiles
NQC = S // 512         # 4 q-chunks
F32 = mybir.dt.float32
# matmul operand dtype: float32r streams at bf16 rate (1 cyc/row) vs 4x for
# plain float32; storage/bytes identical.
USE_F32R = False
BF16 = mybir.dt.bfloat16

_PERM = [0, 4, 1, 5, 2, 6, 3, 7]  # local head order (token-major col blocks)


def _bc(ap, n, where="last"):
    """stride-0 broadcast dim appended (or inserted after partition dim)."""
    if where == "last":
        return bass.AP(tensor=ap.tensor, offset=ap.offset, ap=[*ap.ap, [0, n]])
    return bass.AP(tensor=ap.tensor, offset=ap.offset,
                   ap=[ap.ap[0], [0, n], *ap.ap[1:]])


def classify_mask(mask):
    """Per (qc, kt) block class for scores^T blocks.
    Returns (classes[NQC][kt] in {'skip','clean',int mask-tile-idx}, tiles)."""
    classes = []
    tiles = []
    keyidx = {}
    for qc in range(NQC):
        row = []
        for kt in range(NT):
            sub = mask[qc * 512:(qc + 1) * 512, kt * 128:(kt + 1) * 128]
            if sub.all():
                row.append("skip")
            elif not sub.any():
                row.append("clean")
            else:
                t = (~sub.T).astype(np.float32)  # [128k, 512q] 1=keep
                key = t.tobytes()
                if key not in keyidx:
                    keyidx[key] = len(tiles)
                    tiles.append(t)
                row.append(keyidx[key])
        classes.append(row)
    return classes, tiles


def build_program(classes, n_masks):
    nc = bacc.Bacc("TRN2", target_bir_lowering=False, debug=False)

    def mm(out, lhsT, rhs, start, stop):
        if USE_F32R and lhsT.dtype == F32:
            lhsT = lhsT.bitcast(mybir.dt.float32r)
            rhs = rhs.bitcast(mybir.dt.float32r)
        nc.tensor.matmul(out, lhsT=lhsT, rhs=rhs, start=start, stop=stop)

    xT = nc.dram_tensor("xT", [D, S], BF16, kind="ExternalInput")
    wq = nc.dram_tensor("wq", [D, 512], BF16, kind="ExternalInput")
    wkv = nc.dram_tensor("wkv", [D, 256], BF16, kind="ExternalInput")
    wg = nc.dram_tensor("wg", [D, 512], BF16, kind="ExternalInput")
    wo = nc.dram_tensor("wo", [512, D], BF16, kind="ExternalInput")
    cosq = nc.dram_tensor("cosq", [S, HD], F32, kind="ExternalInput")
    sinq = nc.dram_tensor("sinq", [S, HD], F32, kind="ExternalInput")
    cosk = nc.dram_tensor("cosk", [S, HD], F32, kind="ExternalInput")
    sink = nc.dram_tensor("sink", [S, HD], F32, kind="ExternalInput")
    qsc = nc.dram_tensor("qsc", [128, 512], F32, kind="ExternalInput")
    ksc = nc.dram_tensor("ksc", [128, 128], F32, kind="ExternalInput")
    if n_masks:
        maskt = nc.dram_tensor("maskt", [n_masks, 128, 512], BF16,
                               kind="ExternalInput")
    y = nc.dram_tensor("y", [S, D], F32, kind="ExternalOutput")
    gs_dram = nc.dram_tensor("gs_scratch", [512, S], F32)

    ident_np_name = nc.dram_tensor("ident", [128, 128], F32,
                                   kind="ExternalInput")

    from contextlib import ExitStack
    with tile.TileContext(nc) as tc, ExitStack() as es:
        singles = es.enter_context(tc.tile_pool(name="singles", bufs=1))
        xpool = es.enter_context(tc.tile_pool(name="xpool", bufs=2))
        pwork = es.enter_context(tc.tile_pool(name="pwork", bufs=3))
        psum = es.enter_context(tc.tile_pool(name="psum", bufs=1, space="PSUM"))
        awork = es.enter_context(tc.tile_pool(name="awork", bufs=3, space="SBUF"))

        # ---- resident constants / weights ----
        wq_sb = singles.tile([128, NT, 512], BF16)
        nc.sync.dma_start(out=wq_sb, in_=wq.ap().rearrange("(a p) n -> p a n", p=128))
        wkv_sb = singles.tile([128, NT, 256], BF16)
        nc.sync.dma_start(out=wkv_sb, in_=wkv.ap().rearrange("(a p) n -> p a n", p=128))
        wg_sb = singles.tile([128, NT, 512], BF16)
        nc.sync.dma_start(out=wg_sb, in_=wg.ap().rearrange("(a p) n -> p a n", p=128))
        wo_sb = singles.tile([128, 4, D], BF16)
        nc.sync.dma_start(out=wo_sb, in_=wo.ap().rearrange("(a p) n -> p a n", p=128))
        cosq_sb = singles.tile([128, NT, HD], F32)
        nc.sync.dma_start(out=cosq_sb, in_=cosq.ap().rearrange("(a p) n -> p a n", p=128))
        sinq_sb = singles.tile([128, NT, HD], F32)
        nc.sync.dma_start(out=sinq_sb, in_=sinq.ap().rearrange("(a p) n -> p a n", p=128))
        cosk_sb = singles.tile([128, NT, HD], F32)
        nc.sync.dma_start(out=cosk_sb, in_=cosk.ap().rearrange("(a p) n -> p a n", p=128))
        sink_sb = singles.tile([128, NT, HD], F32)
        nc.sync.dma_start(out=sink_sb, in_=sink.ap().rearrange("(a p) n -> p a n", p=128))
        qsc_sb = singles.tile([128, 512], F32)
        nc.sync.dma_start(out=qsc_sb, in_=qsc.ap())
        ksc_sb = singles.tile([128, 128], F32)
        nc.sync.dma_start(out=ksc_sb, in_=ksc.ap())
        ident_sb = singles.tile([128, 128], F32)
        nc.sync.dma_start(out=ident_sb, in_=ident_np_name.ap())
        if n_masks:
            mask_sb = singles.tile([128, n_masks, 512], BF16)
            nc.sync.dma_start(out=mask_sb,
                              in_=maskt.ap().rearrange("a p n -> p a n"))

        qT = singles.tile([128, 4, S], BF16)       # head nt @0-63, 4+nt @64-127
        kT = singles.tile([128, S], BF16)          # group0 @0-63, group1 @64-127
        vext = singles.tile([128, 2, NT, 65], BF16)  # [v(64) | ones]
        nc.vector.memset(vext[:, :, :, 64], 1.0)
        eps_sb = singles.tile([128, 1], F32)
        nc.vector.memset(eps_sb, float(EPS))
        ones_sb = singles.tile([128, 64], BF16)
        nc.vector.memset(ones_sb, 1.0)

        # ================= Phase P: projections, norm, rope, transpose ====
        for i in range(NT):
            xt = xpool.tile([128, NT, 128], BF16, tag="xt")
            nc.sync.dma_start(
                out=xt, in_=xT.ap()[:, i * 128:(i + 1) * 128]
                .rearrange("(a p) m -> p a m", p=128))

            q_ps = psum.tile([128, 512], mybir.dt.float32, tag="ps_a", bufs=3, name=f"qps_{i}")
            for dt_ in range(NT):
                mm(q_ps, xt[:, dt_, :], rhs=wq_sb[:, dt_, :],
                                 start=(dt_ == 0), stop=(dt_ == NT - 1))
            kv_ps = psum.tile([128, 256], mybir.dt.float32, tag="ps_b", bufs=2, name=f"kvps_{i}")
            for dt_ in range(NT):
                mm(kv_ps, xt[:, dt_, :], rhs=wkv_sb[:, dt_, :],
                                 start=(dt_ == 0), stop=(dt_ == NT - 1))
            # gate^T feature-major [n, s-tile]
            for nt in range(4):
                g_ps = psum.tile([128, 128], mybir.dt.float32, tag="ps_c", bufs=2, name=f"gps_{i}_{nt}")
                for dt_ in range(NT):
                    mm(g_ps, wg_sb[:, dt_, nt * 128:(nt + 1) * 128],
                        rhs=xt[:, dt_, :],
                        start=(dt_ == 0), stop=(dt_ == NT - 1))
                gsig = pwork.tile([128, 128], F32, tag="gsig")
                nc.scalar.activation(gsig, g_ps,
                                     mybir.ActivationFunctionType.Sigmoid)
                nc.sync.dma_start(
                    out=gs_dram.ap()[nt * 128:(nt + 1) * 128,
                                     i * 128:(i + 1) * 128],
                    in_=gsig)

            # ---- q rmsnorm + rope (token-major) ----
            qf = pwork.tile([128, 8, 64], F32, tag="qf")
            rot = pwork.tile([128, 8, 64], F32, tag="rot")
            sq = pwork.tile([128, 8, 64], F32, tag="sq")
            ss = pwork.tile([128, 8], F32, tag="ss")
            q3 = q_ps.rearrange("p (h e) -> p h e", e=64)
            nc.scalar.square(sq, q3)
            nc.vector.reduce_sum(ss, sq, axis=mybir.AxisListType.X)
            nc.scalar.activation(ss, ss, mybir.ActivationFunctionType.Sqrt,
                                 bias=eps_sb, scale=1.0 / 64)
            nc.vector.reciprocal(ss, ss)
            # qhat = q * rstd * (1+q_scale)  (reuse sq as staging)
            for h in range(8):
                nc.vector.tensor_scalar_mul(sq[:, h, :], q3[:, h, :],
                                            ss[:, h:h + 1])
            nc.vector.tensor_mul(sq.rearrange("p h e -> p (h e)"),
                                 sq.rearrange("p h e -> p (h e)"), qsc_sb)
            nc.vector.tensor_scalar_mul(rot[:, :, 0:32], sq[:, :, 32:64], -1.0)
            nc.vector.tensor_copy(rot[:, :, 32:64], sq[:, :, 0:32])
            for h in range(8):
                nc.vector.tensor_mul(qf[:, h, :], sq[:, h, :], cosq_sb[:, i, :])
                nc.vector.tensor_mul(rot[:, h, :], rot[:, h, :], sinq_sb[:, i, :])
            nc.vector.tensor_add(qf.rearrange("p h e -> p (h e)"),
                                 qf.rearrange("p h e -> p (h e)"),
                                 rot.rearrange("p h e -> p (h e)"))

            # ---- k rmsnorm + rope ----
            kf = pwork.tile([128, 2, 64], F32, tag="kf")
            krot = pwork.tile([128, 2, 64], F32, tag="krot")
            ksq = pwork.tile([128, 2, 64], F32, tag="ksq")
            kss = pwork.tile([128, 2], F32, tag="kss")
            k3 = kv_ps[:, 0:128].rearrange("p (h e) -> p h e", e=64)
            nc.scalar.square(ksq, k3)
            nc.vector.reduce_sum(kss, ksq, axis=mybir.AxisListType.X)
            nc.scalar.activation(kss, kss, mybir.ActivationFunctionType.Sqrt,
                                 bias=eps_sb, scale=1.0 / 64)
            nc.vector.reciprocal(kss, kss)
            for h in range(2):
                nc.vector.tensor_scalar_mul(ksq[:, h, :], k3[:, h, :],
                                            kss[:, h:h + 1])
            nc.vector.tensor_mul(ksq.rearrange("p h e -> p (h e)"),
                                 ksq.rearrange("p h e -> p (h e)"), ksc_sb)
            nc.vector.tensor_scalar_mul(krot[:, :, 0:32], ksq[:, :, 32:64], -1.0)
            nc.vector.tensor_copy(krot[:, :, 32:64], ksq[:, :, 0:32])
            for h in range(2):
                nc.vector.tensor_mul(kf[:, h, :], ksq[:, h, :], cosk_sb[:, i, :])
                nc.vector.tensor_mul(krot[:, h, :], krot[:, h, :], sink_sb[:, i, :])
            nc.vector.tensor_add(kf.rearrange("p h e -> p (h e)"),
                                 kf.rearrange("p h e -> p (h e)"),
                                 krot.rearrange("p h e -> p (h e)"))

            # v into v_ext (cast to MMDT)
            for g in range(2):
                nc.vector.tensor_copy(
                    vext[:, g, i, 0:64],
                    kv_ps[:, 128 + g * 64:128 + (g + 1) * 64])

            # ---- transposes to feature-major ----
            qf2 = qf.rearrange("p h e -> p (h e)")
            for nt in range(4):
                tp = psum.tile([128, 128], mybir.dt.float32, tag="ps_d", bufs=1, name=f"tp_{i}_{nt}")
                nc.tensor.transpose(tp, qf2[:, nt * 128:(nt + 1) * 128], ident_sb)
                nc.vector.tensor_copy(qT[:, nt, i * 128:(i + 1) * 128], tp)
            kf2 = kf.rearrange("p h e -> p (h e)")
            tpk = psum.tile([128, 128], mybir.dt.float32, tag="ps_d", bufs=1, name=f"tpk_{i}")
            nc.tensor.transpose(tpk, kf2, ident_sb)
            nc.vector.tensor_copy(kT[:, i * 128:(i + 1) * 128], tpk)

        # ================= Phase A: attention + output projection ========
        for qc in range(NQC):
            ctxg = [awork.tile([128, 512], BF16, tag=f"ctxg{nt}",
                                name=f"ctxg{nt}_{qc}", bufs=2)
                    for nt in range(4)]
            for h in (0, 4, 1, 5, 2, 6, 3, 7):
                g, nt = h // 4, h % 4
                base = 64 * g
                q_rhs = qT[base:base + 64, nt, qc * 512:(qc + 1) * 512]
                ctx_ps = psum.tile([128, 512], mybir.dt.float32, tag="ps_b", bufs=2, name=f"ctx_{qc}_{h}")
                kts = [kt for kt in range(NT) if classes[qc][kt] != "skip"]
                for j, kt in enumerate(kts):
                    s_ps = psum.tile([128, 512], mybir.dt.float32, tag="ps_a", bufs=3, name=f"sps_{qc}_{h}_{kt}")
                    mm(s_ps, kT[base:base + 64, kt * 128:(kt + 1) * 128],
                        rhs=q_rhs, start=True, stop=True)
                    eT = awork.tile([128, 512], BF16, tag="eT")
                    nc.scalar.activation(eT, s_ps,
                                         mybir.ActivationFunctionType.Exp)
                    cls = classes[qc][kt]
                    if cls != "clean":
                        w = min(512, (kt + 1) * 128 - qc * 512)
                        nc.vector.tensor_mul(eT[:, 0:w], eT[:, 0:w],
                                             mask_sb[:, cls, 0:w])
                    mm(ctx_ps[0:65, :], vext[:, g, kt, :],
                                     rhs=eT, start=(j == 0),
                                     stop=(j == len(kts) - 1))
                # normalize + gate
                rstage = awork.tile([65, 512], BF16, tag="rstage", bufs=2)
                with nc.allow_low_precision(reason="bf16 softmax denom"):
                    nc.vector.reciprocal(rstage[64:65, :], ctx_ps[64:65, :])
                rb_ps = psum.tile([64, 512], mybir.dt.float32, tag="ps_d",
                                  bufs=1, name=f"rbps_{qc}_{h}")
                mm(rb_ps, ones_sb[64:65, :], rhs=rstage[64:65, :],
                   start=True, stop=True)
                rb = awork.tile([64, 512], F32, tag="rb", bufs=2)
                nc.vector.tensor_copy(rb, rb_ps)
                gst = awork.tile([64, 512], F32, tag="gst", bufs=2)
                nc.sync.dma_start(
                    out=gst,
                    in_=gs_dram.ap()[128 * nt + 64 * g:128 * nt + 64 * g + 64,
                                     qc * 512:(qc + 1) * 512])
                tmp = awork.tile([64, 512], F32, tag="tmpn", bufs=2)
                nc.vector.tensor_mul(tmp, ctx_ps[0:64, :], rb)
                if g == 0:
                    nc.vector.tensor_mul(ctxg[nt][0:64, :], tmp, gst)
                else:
                    tmp2 = awork.tile([64, 512], BF16, tag="tmp2", bufs=2)
                    nc.vector.tensor_mul(tmp2, tmp, gst)
                    nc.sync.dma_start(out=ctxg[nt][64:128, :], in_=tmp2)

            # output projection for this q-chunk
            for ssub in range(4):
                srow = qc * 512 + ssub * 128
                ostage = awork.tile([128, D], F32, tag="ostage", bufs=2)
                for dc in range(4):
                    o_ps = psum.tile([128, 512], mybir.dt.float32, tag="ps_c", bufs=2, name=f"ops_{qc}_{ssub}_{dc}")
                    for nt in range(4):
                        mm(o_ps, ctxg[nt][:, ssub * 128:(ssub + 1) * 128],
                            rhs=wo_sb[:, nt, dc * 512:(dc + 1) * 512],
                            start=(nt == 0), stop=(nt == 3))
                    nc.scalar.copy(ostage[:, dc * 512:(dc + 1) * 512], o_ps)
                nc.sync.dma_start(out=y.ap()[srow:srow + 128, :], in_=ostage)

    nc.compile()
    return nc


def _bc_part(ap, n):
    """partition-broadcast AP: [1, F] row -> [n, F] (stride-0 partition)."""
    return bass.AP(tensor=ap.tensor, offset=ap.offset,
                   ap=[[0, n], *ap.ap[1:]])


_CACHE = {}


def _prep_core_inputs(inputs, b, t):
    x = inputs["x"]
    Wq, Wk, Wv, Wg, Wo = (inputs[k] for k in ("Wq", "Wk", "Wv", "Wg", "Wo"))
    q_scale, k_scale = inputs["q_scale"], inputs["k_scale"]
    cos, sin = inputs["cos"], inputs["sin"]

    heads = [8 * t + p for p in _PERM]
    qcols = np.concatenate([np.arange(h * 64, (h + 1) * 64) for h in heads])
    groups = [2 * t, 2 * t + 1]
    kcols = np.concatenate([np.arange(g * 64, (g + 1) * 64) for g in groups])

    import ml_dtypes
    bf = ml_dtypes.bfloat16
    xT = np.ascontiguousarray(x[b].T).astype(bf)
    wq = np.ascontiguousarray(Wq[:, qcols]).astype(bf)
    wkv = np.ascontiguousarray(
        np.concatenate([Wk[:, kcols], Wv[:, kcols]], axis=1)).astype(bf)
    wg = np.ascontiguousarray(Wg[:, qcols]).astype(bf)
    wo = np.ascontiguousarray(Wo[qcols, :]).astype(bf)
    scaling = float(HD) ** -0.5
    d = {
        "xT": xT, "wq": wq, "wkv": wkv, "wg": wg, "wo": wo,
        "cosq": (cos * scaling).astype(np.float32),
        "sinq": (sin * scaling).astype(np.float32),
        "cosk": cos.astype(np.float32), "sink": sin.astype(np.float32),
        "qsc": np.broadcast_to(
            np.tile(1.0 + q_scale, 8)[None, :], (128, 512)).copy(),
        "ksc": np.broadcast_to(
            np.tile(1.0 + k_scale, 2)[None, :], (128, 128)).copy(),
        "ident": np.eye(128, dtype=np.float32),
    }
    return d


def kernel(**inputs):
    mask = np.asarray(inputs["mask"])
    classes, tiles = classify_mask(mask)
    key = mask.tobytes()
    if key not in _CACHE:
        _CACHE[key] = (build_program(classes, len(tiles)), classes, tiles)
    nc, classes, tiles = _CACHE[key]

    import ml_dtypes
    mask_arr = (np.stack(tiles).astype(ml_dtypes.bfloat16) if tiles
                else np.zeros((0, 128, 512), np.float32))
    in_maps = []
    for c in range(NCORES):
        m = _prep_core_inputs(inputs, c // 4, c % 4)
        if len(tiles):
            m["maskt"] = mask_arr
        in_maps.append(m)

    res = run_bass_kernel_spmd(nc, in_maps, list(range(NCORES)))
    out = np.zeros((B, S, D), np.float32)
    for c in range(NCORES):
        out[c // 4] += res.results[c]["y"]
    return out.astype(inputs["x"].dtype)

